# revision 36
# baseline (speedup 1.0000x reference)
"""Trainium2 Bass kernel for BinaryCE + rejection-softmax loss.

Reference computation (B=256, C=500, D=256):
    y = labels.astype(f32)                                   # [B, C]
    bce[b] = sum_c( softplus(logits) - y*logits )            # log-sigmoid BCE
    max_sim[b, c] = max_d wf[c, b, d]
    rej[b] = sum_c (labels==0) * relu(sigmoid(max_sim) - 0.3)
    out[b] = bce[b] + rej[b]

Sharding: data-parallel over B across 8 cores (wf on axis 1,
logits/labels on axis 0). Per core: logits [32,500], wf [500,32,256],
labels [32,500] -> out [32]. No cross-device reduction.

Strategy notes (measured on HW):
- wf is cast to fp16 on the host: max/sigmoid tolerate ~5e-4 rel error
  vs the 2e-2 gate, and the wf stream is the roofline.
- Layout [128 partitions, 32768]: partition p holds classes 4p..4p+3
  (500 padded to 512 — 125-partition DMAs break the SWDGE 16-lane
  descriptor waves into 5 data + 11 dummy packets, 40% slower).
- The SWDGE queue issues ~24-39ns per descriptor, so descriptor size
  sets the stream rate (16 KB -> ~414 GB/s, 8 KB -> ~310, 4 KB ->
  ~130). The whole stream goes through the single SWDGE queue as
  32-row chunks (16 KB descriptors) with two 16-row tail chunks to
  shorten the post-stream drain. The HWDGE rings measured 47-88 GB/s
  here and slow the SWDGE to ~175 GB/s when mixed in — single queue
  wins. DMA accum (cce_op) does not support max, so the DMA cannot
  pre-combine.
- max over D runs as a tensor_tensor max tree (fp16 hits the DVE 2x
  perf mode; TensorReduce/InstPool have no fast mode, and gpsimd
  rejects TENSOR_TENSOR outright): per-chunk 256->128, then per
  32-row class group 128->64->32->16 TTs and a 16-wide reduce. The
  rejection tail per group is sigmoid (ACT) then one DVE
  scalar_tensor_tensor mask*max(sig, 0.3) — the +0.3*sum(mask)
  overcount is cancelled by early -0.3*ones x mask matmuls — then a
  ones-matmul accumulate into PSUM [1, 32] next to the BCE row
  (injected via an identity-matmul transpose). BCE and the label
  mask build run in the DMA-spinup shadow.
"""

import sys

for _p in ("/root/.axon_site", "/root/.axon_site/_ro/trn_rl_repo",
           "/root/.axon_site/_ro/pypackages", "/opt/trn_rl_repo"):
    if _p not in sys.path:
        sys.path.append(_p)

import numpy as np

import concourse.bass as bass  # noqa: F401  (registers engine classes)
import concourse.tile as tile
from concourse import bacc, mybir
from concourse.bass_utils import run_bass_kernel_spmd
from concourse.masks import make_identity

F32 = mybir.dt.float32
F16 = mybir.dt.float16
I32 = mybir.dt.int32
AF = mybir.ActivationFunctionType
ALU = mybir.AluOpType
AX = mybir.AxisListType

B, C, D = 256, 500, 256
REJECTION_MARGIN = 0.3
NCORES = 8
BL = B // NCORES          # 32 samples per core
CPP = 128                 # class-partitions: 4 classes each (512 padded)
CUSED = 125               # partitions holding real classes
C4 = 4
ROWS = C4 * BL            # 128 (c4, b) rows per partition

# (row0, nrows, queue): queue g=gpsimd/SWDGE (s=sync ring and a=scalar
# ring measured far slower and interfere; unused). 32-row chunks give
# 16 KB descriptors; the two 16-row tail chunks shorten the drain.
CHUNKS = [
    (0, 32, "g"),
    (32, 32, "g"),
    (64, 32, "g"),
    (96, 16, "g"),
    (112, 16, "g"),
]
# TT2..TT4/reduce spans: one per 32-row class group so each rejection
# tail fires as soon as its msim rows complete.
SPANS = [(g * BL, BL) for g in range(C4)]


def build_nc(debug: bool = False):
    nc = bacc.Bacc("TRN2", target_bir_lowering=False, debug=debug)

    logits_d = nc.dram_tensor("logits", [BL, C], F32, kind="ExternalInput")
    wf_d = nc.dram_tensor("wf", [CPP, ROWS * D], F16, kind="ExternalInput")
    labels_d = nc.dram_tensor("labels", [BL, C], F32, kind="ExternalInput")
    out_d = nc.dram_tensor("out", [1, BL], F32, kind="ExternalOutput")

    wfv = wf_d[:]
    queues = {"g": nc.gpsimd, "s": nc.sync, "a": nc.scalar}

    with tile.TileContext(nc) as tc:
        with (
            tc.tile_pool(name="consts", bufs=1) as consts,
            tc.tile_pool(name="wfp", bufs=len(CHUNKS)) as wfp,
            tc.tile_pool(name="tailp", bufs=2) as tailp,
            tc.tile_pool(name="psum_t", bufs=2, space="PSUM") as psum_t,
            tc.tile_pool(name="psum_acc", bufs=1, space="PSUM") as psum_acc,
        ):
            # --- small inputs on the ACT ring (tiny, first in queue) --------
            logits_sb = consts.tile([BL, C], F32)
            nc.scalar.dma_start(logits_sb[:], logits_d[:])
            labels_f = consts.tile([BL, C], F32)
            nc.scalar.dma_start(labels_f[:], labels_d[:])

            # --- wf stream across three queues -----------------------------
            wfts = []
            for r0, R, qn in CHUNKS:
                wft = wfp.tile([CPP, 32 * D], F16, tag="wft")
                queues[qn].dma_start(wft[:, :R * D], wfv[:, r0 * D:(r0 + R) * D])
                wfts.append((wft, r0, R))

            # gpsimd helpers after the descgens (program order matters there)
            ident = consts.tile([BL, BL], F32)
            make_identity(nc, ident[:])

            ones = consts.tile([CPP, 1], F32)
            nc.gpsimd.memset(ones[:], 1.0)
            neg_ones = consts.tile([CPP, 1], F32)
            nc.gpsimd.memset(neg_ones[:], -REJECTION_MARGIN)

            # --- BCE part in natural [b, c] layout (DMA-spinup shadow) -----
            # softplus(x) = ln(exp(x) + 1); no Softplus LUT on TRN2.
            # Safe: |logits| <~ 5 so exp() cannot overflow.
            exp_tmp = consts.tile([BL, C], F32)
            nc.scalar.activation(exp_tmp[:], logits_sb[:], AF.Exp)
            sp_tmp = consts.tile([BL, C], F32)
            sp_sum = consts.tile([BL, 1], F32)
            nc.scalar.activation(sp_tmp[:], exp_tmp[:], AF.Ln, bias=1.0,
                                 accum_out=sp_sum[:])
            yx_tmp = consts.tile([BL, C], F32)
            yx_sum = consts.tile([BL, 1], F32)
            nc.vector.tensor_mul(yx_tmp[:], labels_f[:], logits_sb[:])
            nc.vector.reduce_sum(yx_sum[:], yx_tmp[:], axis=AX.X)
            bce_col = consts.tile([BL, 1], F32)
            nc.vector.tensor_sub(bce_col[:], sp_sum[:], yx_sum[:])

            # --- mask = 1 - labels^T in [p, c4, b] layout (c = 4p + c4) ----
            # Padded classes c >= 500 (partitions >= 125) keep mask 0 from
            # the memset, so the zero-padded wf rows contribute nothing.
            mask_sb = consts.tile([CPP, C4, BL], F32)
            nc.gpsimd.memset(mask_sb[:], 0.0)
            for g in range(C4):
                labT = psum_t.tile([CUSED, BL], F32, tag="labT")
                nc.tensor.matmul(labT[:], labels_f[:, g::C4], ident[:],
                                 start=True, stop=True)
                nc.scalar.activation(mask_sb[:CUSED, g, :], labT[:],
                                     AF.Identity, bias=1.0, scale=-1.0)

            # --- PSUM accumulator [1, 32]; BCE row first -------------------
            acc = psum_acc.tile([1, BL], F32)
            nc.tensor.matmul(acc[:], bce_col[:], ident[:],
                             start=True, stop=False)

            msim = consts.tile([CPP, ROWS], F16)
            t1_all = consts.tile([CPP, ROWS, 128], F16)

            # mask * relu(sig - 0.3) summed == mask * max(sig, 0.3) summed
            # minus 0.3 * sum(mask); the correction is accumulated early
            # via the -0.3*ones x mask matmuls (off the critical tail).
            for g in range(C4):
                nc.tensor.matmul(acc[:], neg_ones[:], mask_sb[:, g, :],
                                 start=False, stop=False)

            def tail(g):
                sig = tailp.tile([CPP, BL], F32, tag="sig")
                nc.scalar.activation(sig[:], msim[:, BL * g:BL * g + BL],
                                     AF.Sigmoid)
                rejm = tailp.tile([CPP, BL], F32, tag="rejm")
                nc.vector.scalar_tensor_tensor(
                    rejm[:], sig[:], REJECTION_MARGIN, mask_sb[:, g, :],
                    op0=ALU.max, op1=ALU.mult)
                nc.tensor.matmul(acc[:], ones[:], rejm[:],
                                 start=False, stop=(g == C4 - 1))

            # --- max tree: per-chunk 256->128 (fp16 2x TT), then per-span
            # 128->64->32 TTs and a 32-wide reduce into msim ----------------
            for k, (wft, r0, R) in enumerate(wfts):
                w3 = wft[:, :R * D].rearrange("p (r d) -> p r d", d=D)
                nc.vector.tensor_max(t1_all[:, r0:r0 + R, :],
                                     w3[:, :, 0:128], w3[:, :, 128:256])

            # spans fire in row order; each waits on the t1 rows it reads
            for s0, S in SPANS:
                t2 = tailp.tile([CPP, BL, 64], F16, tag="t2")
                nc.vector.tensor_max(t2[:, :S, :],
                                     t1_all[:, s0:s0 + S, 0:64],
                                     t1_all[:, s0:s0 + S, 64:128])
                t3 = tailp.tile([CPP, BL, 32], F16, tag="t3")
                nc.vector.tensor_max(t3[:, :S, :],
                                     t2[:, :S, 0:32], t2[:, :S, 32:64])
                t4 = tailp.tile([CPP, BL, 16], F16, tag="t4")
                nc.vector.tensor_max(t4[:, :S, :],
                                     t3[:, :S, 0:16], t3[:, :S, 16:32])
                nc.vector.reduce_max(msim[:, s0:s0 + S], t4[:, :S, :],
                                     axis=AX.X)
                if (s0 + S) % BL == 0:
                    tail((s0 + S) // BL - 1)

            out_sb = consts.tile([1, BL], F32)
            nc.scalar.copy(out_sb[:], acc[:])
            nc.scalar.dma_start(out_d[:], out_sb[:])

    nc.compile()
    return nc


_NC_CACHE = None


def _get_nc():
    global _NC_CACHE
    if _NC_CACHE is None:
        _NC_CACHE = build_nc()
    return _NC_CACHE


def _in_maps(logits, wf, labels):
    maps = []
    for k in range(NCORES):
        b0 = k * BL
        wf16 = np.zeros((CPP, ROWS * D), dtype=np.float16)
        wf16[:CUSED] = np.ascontiguousarray(
            wf[:, b0:b0 + BL, :]).astype(np.float16).reshape(CUSED, ROWS * D)
        maps.append({
            "logits": np.ascontiguousarray(logits[b0:b0 + BL]),
            "wf": wf16,
            "labels": np.ascontiguousarray(
                labels[b0:b0 + BL]).astype(np.float32),
        })
    return maps


def run(logits, wf, labels, trace: bool = False, tmpdir: str | None = None):
    """Run on all 8 cores; returns (full_output [B], BassKernelResults)."""
    logits = np.asarray(logits, dtype=np.float32)
    wf = np.asarray(wf, dtype=np.float32)
    labels = np.asarray(labels, dtype=np.int32)
    assert logits.shape == (B, C) and wf.shape == (C, B, D) \
        and labels.shape == (B, C)

    nc = _get_nc()
    res = run_bass_kernel_spmd(nc, _in_maps(logits, wf, labels),
                               list(range(NCORES)), trace=trace,
                               tmpdir=tmpdir)
    out = np.concatenate(
        [np.asarray(res.results[k]["out"]).reshape(BL) for k in range(NCORES)])
    return out.astype(np.float32), res


def kernel(logits, wf, labels):
    out, _ = run(logits, wf, labels)
    return out
